# revision 23
# baseline (speedup 1.0000x reference)
"""NodeAttention Trainium2 kernel (per-core program, SPMD over 8 cores).

v2 strategy (per core, i-block of NI=96 query rows; j on partitions):
- pair data arrives host-transposed+cast: PT [C, NJB, NI, 128] bf16 so the
  per-jb DMA is one contiguous 24.5KB run per partition (line-rate HBM).
- LN+bias projection folded into one [128 -> 9] matmul per i with the
  mean-correction folded into the weights host-side:
    W'_ch = lnw_c*wb_ch - s_h/C  (s_h = sum_c lnw_c*wb_ch), col 8 = 1/C.
  The per-head additive constant t_h is dropped (softmax-invariant).
  logits = sim + r * dots',  r = rsqrt(meansq - mu^2 + eps).
- sumsq via DVE square (bf16 2x) + per-i ones-matmul; dots/ss matmuls write
  [j, i, {dots|ss}] PSUM directly (FWL-eligible 128-col bf16 stationaries).
- softmax without max-subtraction; normalizer via ones column in V;
  attention accumulates in PSUM across all 6 j-blocks.
"""
import numpy as np
from contextlib import ExitStack

import concourse.bass as bass
import concourse.tile as tile
from concourse import mybir
from concourse.masks import make_identity

f32 = mybir.dt.float32
bf16 = mybir.dt.bfloat16
u8 = mybir.dt.uint8

N = 768          # sequence length (j axis, also full i)
C = 128          # pair channels
H = 8            # heads
D = 32           # head dim
INNER = 256      # H*D
ND = 256         # node dim
NJB = N // 128   # 6 j-blocks
EPS = 1e-5
CH = 48          # i-half chunk (per-jb dots PSUM bank = 48*10*4 = 1920B)


def _bcast_h(ap2d: bass.AP, h: int) -> bass.AP:
    """[P, F] -> [P, h, F] with step-0 broadcast over the middle dim."""
    ap = list(ap2d.ap)
    assert len(ap) == 2
    return bass.AP(ap2d.tensor, ap2d.offset, [ap[0], [0, h], ap[1]])


def _view(ap_t: bass.AP, frees, off=0) -> bass.AP:
    """Rebuild an AP keeping partition dim, with explicit free [step, num]s."""
    ap = list(ap_t.ap)
    return bass.AP(ap_t.tensor, ap_t.offset + off,
                   [ap[0]] + [list(f) for f in frees])


def build_nc(NI=96, n_cores=8, upto='full'):
    nc = bass.Bass("TRN2", target_bir_lowering=False, debug=False,
                   num_devices=n_cores)
    # pair, host-transposed: PT[c, jb, i, jj] = pair[i, jb*128+jj, c]
    pairt = nc.dram_tensor("pairt", [C, NJB, NI, 128], bf16,
                           kind="ExternalInput").ap()
    node = nc.dram_tensor("node", [N, ND], f32, kind="ExternalInput").ap()
    nodeq = nc.dram_tensor("nodeq", [NI, ND], f32, kind="ExternalInput").ap()
    maskq = nc.dram_tensor("maskq", [NI, N], u8, kind="ExternalInput").ap()
    # wext cols 0-7: lnw*wb - s/C (mean-fold), col 8: 1/C (mean for var)
    wext = nc.dram_tensor("wext", [C, 9], bf16, kind="ExternalInput").ap()
    # wnode cols: [Wq*scale | Wk | Wv | Wg]
    wnode = nc.dram_tensor("wnode", [ND, 4 * INNER], bf16, kind="ExternalInput").ap()
    wout = nc.dram_tensor("wout", [INNER, ND], bf16, kind="ExternalInput").ap()
    lnw = nc.dram_tensor("lnw", [1, ND], f32, kind="ExternalInput").ap()
    lnb = nc.dram_tensor("lnb", [1, ND], f32, kind="ExternalInput").ap()
    bg = nc.dram_tensor("bg", [1, INNER], f32, kind="ExternalInput").ap()
    bout = nc.dram_tensor("bout", [1, ND], f32, kind="ExternalInput").ap()
    y_out = nc.dram_tensor("y", [NI, ND], f32, kind="ExternalOutput").ap()
    dbg = nc.dram_tensor("dbg", [128, 4096], f32, kind="ExternalOutput").ap() \
        if upto == 'dbg' else None

    with tile.TileContext(nc) as tc, ExitStack() as ctx:
        const = ctx.enter_context(tc.tile_pool(name="const", bufs=1))
        persist = ctx.enter_context(tc.tile_pool(name="persist", bufs=1))
        tpp = ctx.enter_context(tc.tile_pool(name="tp", bufs=3))

        # ---- TP0 first (jb0 critical path), then small loads, then TP1-5 ----
        TP_tiles = []
        TP0 = tpp.tile([C, NI, 128], bf16, tag="TP", name="TP0")
        nc.sync.dma_start(out=TP0[:], in_=pairt[:, 0])
        TP_tiles.append(TP0)

        # ---- constants ----
        wext_sb = const.tile([C, 9], bf16)
        nc.scalar.dma_start(out=wext_sb[:], in_=wext)
        ones_sb = const.tile([C, 1], bf16)
        nc.vector.memset(ones_sb[:], 1.0)
        ident = const.tile([128, 128], f32)
        make_identity(nc, ident[:])
        ident_bf = const.tile([128, 128], bf16)
        make_identity(nc, ident_bf[:])
        eps_sb = const.tile([128, 1], f32)
        nc.vector.memset(eps_sb[:], EPS)

        def bload(name, src, cols, dtype=f32):
            t = const.tile([128, cols], dtype, name=name)
            src_b = bass.AP(src.tensor, src.offset, [[0, 128]] + list(src.ap)[1:])
            nc.gpsimd.dma_start(out=t[:], in_=src_b)
            return t
        lnw_sb = bload("lnw_sb", lnw, ND)
        lnb_sb = bload("lnb_sb", lnb, ND)
        bg_sb = bload("bg_sb", bg, INNER)
        bout_sb = bload("bout_sb", bout, ND)
        # node-side weights, feat-major tiles [feat%128, feat//128, cols]
        wn_sb = const.tile([128, 2, 4 * INNER], bf16)
        nc.scalar.dma_start(out=wn_sb[:],
                            in_=wnode.rearrange("(kt p) c -> p kt c", p=128))
        wout_sb = const.tile([128, 2, ND], bf16)
        nc.scalar.dma_start(out=wout_sb[:],
                            in_=wout.rearrange("(kt p) c -> p kt c", p=128))

        for jb in range(1, NJB):
            TP = tpp.tile([C, NI, 128], bf16, tag="TP", name=f"TP{jb}")
            nc.sync.dma_start(out=TP[:], in_=pairt[:, jb])
            TP_tiles.append(TP)

        # ---- persistent node-derived tensors ----
        kT_sb = persist.tile([32, H, N], bf16)        # k^T [d, h, j]
        qT_sb = persist.tile([32, H, NI], bf16)       # q^T [d, h, i]
        Vx_sb = persist.tile([128, NJB, H, D + 1], bf16)  # v in [j, jb, h, d|1]
        m01T_sb = persist.tile([128, NJB, NI], bf16)  # mask^T in [j, jb, i]
        sig_sb = persist.tile([NI, INNER], f32)       # sigmoid(g) [i, inner]

        # ---- jb-loop pools (open before preamble so jb0 can start early) --
        NH = NI // CH  # i-half count (2)
        sqp = ctx.enter_context(tc.tile_pool(name="tpsq", bufs=3))
        stp = ctx.enter_context(tc.tile_pool(name="stat", bufs=2))
        lgp = ctx.enter_context(tc.tile_pool(name="logit", bufs=2))
        epp = ctx.enter_context(tc.tile_pool(name="epool", bufs=2))
        accp = ctx.enter_context(tc.tile_pool(name="att_acc", bufs=1))
        dps = ctx.enter_context(tc.tile_pool(name="dots_ps", bufs=2, space="PSUM"))

        att_acc = accp.tile([NI, H, D + 1], f32)
        nc.vector.memset(att_acc[:], 0.0)

        def pair_phase(jb):
            """squares, dots', sumsq, stats, u -> logits[jb] (no node deps)."""
            TP = TP_tiles[jb]
            logits = lgp.tile([128, H, NI], f32, tag="logits", name=f"lg{jb}")
            var = stp.tile([128, NH, CH], f32, tag="var")
            Tsqs, dts = [], []
            for ih in range(NH):
                i0 = ih * CH
                TPh = TP[:, i0:i0 + CH, :]
                Tsq = sqp.tile([C, CH, 128], bf16, tag="sq", name=f"sq{jb}_{ih}")
                nc.vector.tensor_mul(Tsq[:], TPh, TPh)
                Tsqs.append(Tsq)
            for ih in range(NH):
                dt = dps.tile([128, CH, 10], f32, tag=f"dt{ih}", name=f"dt{jb}_{ih}")
                dts.append(dt)
                for il in range(CH):
                    nc.tensor.matmul(dt[:, il, 0:9],
                                     lhsT=TP[:, ih * CH + il, :],
                                     rhs=wext_sb[:])
            for ih in range(NH):
                for il in range(CH):
                    nc.tensor.matmul(dts[ih][:, il, 9:10],
                                     lhsT=Tsqs[ih][:, il, :],
                                     rhs=ones_sb[:])
            for ih in range(NH):
                dt = dts[ih]
                mu_sb = stp.tile([128, CH], f32, tag=f"mu{ih}")
                nc.vector.tensor_copy(mu_sb[:], dt[:, :, 8])
                m2 = stp.tile([128, CH], f32, tag=f"m2{ih}")
                nc.vector.tensor_mul(m2[:], mu_sb[:], mu_sb[:])
                nc.vector.scalar_tensor_tensor(
                    out=var[:, ih, :], in0=dt[:, :, 9], scalar=1.0 / C, in1=m2[:],
                    op0=mybir.AluOpType.mult, op1=mybir.AluOpType.subtract)
            sd = stp.tile([128, NH, CH], f32, tag="sd")
            nc.scalar.activation(sd[:], var[:],
                                 mybir.ActivationFunctionType.Sqrt,
                                 bias=eps_sb[:])
            r = stp.tile([128, NH, CH], f32, tag="r")
            nc.vector.reciprocal(r[:], sd[:])
            for ih in range(NH):
                i0 = ih * CH
                dots_v = _view(dts[ih], [[1, H], [10, CH]])
                r_b = _view(r[:, ih, :], [[0, H], [1, CH]])
                nc.vector.tensor_mul(logits[:, :, i0:i0 + CH], dots_v, r_b)
            return logits

        logits_t = {0: pair_phase(0)}

        # ================= node preamble (overlaps jb0 pair phase) ==========
        with tc.tile_pool(name="nodep", bufs=1) as npool, \
             tc.tile_pool(name="node_ps", bufs=1, space="PSUM") as nps:
            x_all = npool.tile([128, N // 128, ND], f32)
            stats = npool.tile([128, 6], f32)
            mv = npool.tile([128, 2], f32)
            sd = npool.tile([128, 1], f32)
            rln = npool.tile([128, 1], f32)

            def layernorm_tile(xt, nrows):
                nc.vector.bn_stats(out=stats[:nrows, :], in_=xt)
                nc.vector.bn_aggr(out=mv[:nrows, :], in_=stats[:nrows, :])
                nc.scalar.activation(sd[:nrows, :], mv[:nrows, 1:2],
                                     mybir.ActivationFunctionType.Sqrt,
                                     bias=eps_sb[:nrows, :])
                nc.vector.reciprocal(rln[:nrows, :], sd[:nrows, :])
                nc.vector.tensor_scalar(out=xt, in0=xt,
                                        scalar1=mv[:nrows, 0:1],
                                        scalar2=rln[:nrows, :],
                                        op0=mybir.AluOpType.subtract,
                                        op1=mybir.AluOpType.mult)
                nc.vector.tensor_mul(xt, xt, lnw_sb[:nrows, :])
                nc.vector.tensor_add(xt, xt, lnb_sb[:nrows, :])

            for t in range(N // 128):
                nc.scalar.dma_start(out=x_all[:, t, :], in_=node[t * 128:(t + 1) * 128, :])
                layernorm_tile(x_all[:, t, :], 128)

            xq = npool.tile([NI, ND], f32)
            nc.scalar.dma_start(out=xq[:], in_=nodeq)
            layernorm_tile(xq[:], NI)

            # -- transposes: xT [feat, j], xqT [feat, i] --
            xT_sb = npool.tile([128, 2, N], bf16)
            xqT_sb = npool.tile([128, 2, NI], bf16)
            for t in range(N // 128):
                for kt in range(2):
                    tp = nps.tile([128, 128], f32, tag="xpose")
                    nc.tensor.transpose(tp[:], x_all[:, t, kt * 128:(kt + 1) * 128], ident[:])
                    nc.vector.tensor_copy(xT_sb[:, kt, t * 128:(t + 1) * 128], tp[:])
            for kt in range(2):
                tp = nps.tile([128, NI], f32, tag="xpose")
                nc.tensor.transpose(tp[:], xq[:, kt * 128:(kt + 1) * 128], ident[:NI, :NI])
                nc.vector.tensor_copy(xqT_sb[:, kt, :], tp[:])

            # -- k^T = Wk^T @ x^T : per-head [d, j] at partition base 0 --
            for h in range(H):
                for n0 in range(0, N, 384):
                    kp = nps.tile([32, 384], f32, tag="nmm")
                    for kt in range(2):
                        nc.tensor.matmul(
                            kp[:],
                            lhsT=wn_sb[:, kt, INNER + h * D:INNER + (h + 1) * D],
                            rhs=xT_sb[:, kt, n0:n0 + 384],
                            start=(kt == 0), stop=(kt == 1))
                    nc.vector.tensor_copy(kT_sb[:, h, n0:n0 + 384], kp[:])

            # -- v = x @ Wv -> Vx [j, jb, h, d] + ones column --
            for jb in range(NJB):
                vp = nps.tile([128, INNER], f32, tag="vmm")
                for kt in range(2):
                    nc.tensor.matmul(vp[:], lhsT=xT_sb[:, kt, jb * 128:(jb + 1) * 128],
                                     rhs=wn_sb[:, kt, 2 * INNER:3 * INNER],
                                     start=(kt == 0), stop=(kt == 1))
                nc.vector.tensor_copy(Vx_sb[:, jb, :, 0:D],
                                      vp[:].rearrange("p (h d) -> p h d", h=H))
            nc.vector.memset(Vx_sb[:, :, :, D:D + 1], 1.0)

            # -- q^T = (Wq*scale)^T @ xq^T : per-head [d, i] --
            for h in range(H):
                qp = nps.tile([32, NI], f32, tag="nmm")
                for kt in range(2):
                    nc.tensor.matmul(qp[:], lhsT=wn_sb[:, kt, h * D:(h + 1) * D],
                                     rhs=xqT_sb[:, kt, :],
                                     start=(kt == 0), stop=(kt == 1))
                nc.vector.tensor_copy(qT_sb[:, h, :], qp[:])

            # -- g = xq @ Wg + bg ; sig = sigmoid(g) --
            gp = nps.tile([NI, INNER], f32, tag="gmm")
            for kt in range(2):
                nc.tensor.matmul(gp[:], lhsT=xqT_sb[:, kt, :],
                                 rhs=wn_sb[:, kt, 3 * INNER:4 * INNER],
                                 start=(kt == 0), stop=(kt == 1))
            gt = npool.tile([NI, INNER], f32)
            nc.vector.tensor_add(gt[:], gp[:], bg_sb[:NI, :])
            nc.scalar.activation(sig_sb[:NI, :], gt[:],
                                 mybir.ActivationFunctionType.Sigmoid)

            # -- mask: [i, j] u8 -> m01T [j, jb, i] bf16 --
            mu8 = npool.tile([NI, N], u8)
            nc.scalar.dma_start(out=mu8[:], in_=maskq)
            mbf = npool.tile([NI, N], bf16)
            nc.vector.tensor_copy(mbf[:], mu8[:])
            for jb in range(NJB):
                tp = nps.tile([128, NI], bf16, tag="xpose")
                nc.tensor.transpose(tp[:], mbf[:, jb * 128:(jb + 1) * 128],
                                    ident_bf[:NI, :NI])
                nc.vector.tensor_copy(m01T_sb[:, jb, :], tp[:])

        # ============ pipelined jb loop: att/mask delayed one block =========
        E_t, attps_t = {}, {}
        with tc.tile_pool(name="sim_ps", bufs=1, space="PSUM") as simp, \
             tc.tile_pool(name="att_ps", bufs=2, space="PSUM") as attp:
            for k in range(NJB + 1):
                if 1 <= k <= NJB - 1:
                    logits_t[k] = pair_phase(k)
                if k <= NJB - 1:
                    sim_ps = simp.tile([128, H, 128], f32, tag="sim", name=f"sim{k}")
                    for h in range(H):
                        nc.tensor.matmul(
                            sim_ps[:, h, :NI],
                            lhsT=kT_sb[:, h, k * 128:(k + 1) * 128],
                            rhs=qT_sb[:, h, :])
                    lg = logits_t[k]
                    nc.vector.tensor_add(lg[:], lg[:], sim_ps[:, :, :NI])
                    E = epp.tile([128, H, NI], bf16, tag="E", name=f"E{k}")
                    nc.scalar.activation(E[:], lg[:],
                                         mybir.ActivationFunctionType.Exp)
                    E_t[k] = E
                if k >= 1:
                    j = k - 1
                    E = E_t.pop(j)
                    nc.gpsimd.tensor_mul(E[:], E[:], _bcast_h(m01T_sb[:, j, :], H))
                    att_ps = attp.tile([NI, H, D + 1], f32, tag="attjb",
                                       name=f"att{j}")
                    for h in range(H):
                        nc.tensor.matmul(att_ps[:, h, :], lhsT=E[:, h, :],
                                         rhs=Vx_sb[:, j, h, :])
                    attps_t[j] = att_ps
                if k >= 2:
                    ap = attps_t.pop(k - 2)
                    nc.vector.tensor_add(att_acc[:], att_acc[:], ap[:])
            ap = attps_t.pop(NJB - 1)
            nc.vector.tensor_add(att_acc[:], att_acc[:], ap[:])

        # ---- finalize ----
        with tc.tile_pool(name="fin", bufs=1) as fin, \
             tc.tile_pool(name="fin_ps", bufs=2, space="PSUM") as finp:
            den_r = fin.tile([NI, H], f32)
            den_v = _view(att_acc, [[D + 1, H]], off=D)
            nc.vector.reciprocal(den_r[:], den_v)
            att_f = fin.tile([NI, INNER], f32)
            num_v = _view(att_acc, [[D + 1, H], [1, D]])
            den_b = _view(den_r, [[1, H], [0, D]])
            nc.vector.tensor_mul(att_f[:].rearrange("p (h d) -> p h d", h=H),
                                 num_v, den_b)
            gated = fin.tile([NI, INNER], f32)
            nc.vector.tensor_mul(gated[:], att_f[:], sig_sb[:NI, :])

            gT_sb = fin.tile([128, 2, NI], bf16)
            for kt in range(2):
                tp = finp.tile([128, NI], f32, tag="gpose")
                nc.tensor.transpose(tp[:], gated[:, kt * 128:(kt + 1) * 128],
                                    ident[:NI, :NI])
                nc.vector.tensor_copy(gT_sb[:, kt, :], tp[:])

            y_ps = finp.tile([NI, ND], f32, tag="ymm")
            for kt in range(2):
                nc.tensor.matmul(y_ps[:], lhsT=gT_sb[:, kt, :],
                                 rhs=wout_sb[:, kt, :],
                                 start=(kt == 0), stop=(kt == 1))
            y_sb = fin.tile([NI, ND], f32)
            nc.vector.tensor_add(y_sb[:], y_ps[:], bout_sb[:NI, :])
            nc.scalar.dma_start(out=y_out, in_=y_sb[:])

    return nc


def host_prep(inputs, NI=96, n_cores=8):
    """Slice/fold/transpose FULL inputs into per-core in_maps."""
    import ml_dtypes
    node_feats = np.asarray(inputs["node_feats"])[0]      # [N, ND]
    pair_feats = np.asarray(inputs["pair_feats"])[0]      # [N, N, C]
    mask = np.asarray(inputs["mask"])[0]                  # [N, N] bool
    lnw = np.asarray(inputs["ln_node_w"]).reshape(1, ND)
    lnb = np.asarray(inputs["ln_node_b"]).reshape(1, ND)
    lpw = np.asarray(inputs["ln_pair_w"])                 # [C]
    lpb = np.asarray(inputs["ln_pair_b"])                 # [C]  (t_h drops out)
    w_qkv = np.asarray(inputs["w_qkv"])                   # [ND, 3*INNER]
    w_g = np.asarray(inputs["w_g"])                       # [ND, INNER]
    b_g = np.asarray(inputs["b_g"]).reshape(1, INNER)
    w_bias = np.asarray(inputs["w_bias"])                 # [C, H]
    w_out = np.asarray(inputs["w_out"])                   # [INNER, ND]
    b_out = np.asarray(inputs["b_out"]).reshape(1, ND)

    Wp = (lpw[:, None] * w_bias).astype(np.float32)       # [C, H]
    s_h = Wp.sum(0)                                       # [H]
    wext = np.zeros((C, 9), np.float32)
    wext[:, 0:H] = Wp - s_h[None, :] / C                  # mean-fold
    wext[:, 8] = 1.0 / C
    wext = wext.astype(ml_dtypes.bfloat16)

    scale = D ** -0.5
    wnode = np.concatenate([w_qkv[:, 0:INNER] * scale,
                            w_qkv[:, INNER:2 * INNER],
                            w_qkv[:, 2 * INNER:3 * INNER],
                            w_g], axis=1).astype(ml_dtypes.bfloat16)
    woutb = w_out.astype(ml_dtypes.bfloat16)

    shared = dict(node=node_feats.astype(np.float32), wext=wext, wnode=wnode,
                  wout=woutb, lnw=lnw.astype(np.float32), lnb=lnb.astype(np.float32),
                  bg=b_g.astype(np.float32), bout=b_out.astype(np.float32))
    in_maps = []
    for c in range(n_cores):
        i0 = c * NI
        q = pair_feats[i0:i0 + NI].transpose(2, 0, 1)     # [C, NI, N]
        pt = q.reshape(C, NI, NJB, 128).transpose(0, 2, 1, 3)  # [C, jb, i, jj]
        in_maps.append(dict(
            pairt=np.ascontiguousarray(pt).astype(ml_dtypes.bfloat16),
            nodeq=np.ascontiguousarray(node_feats[i0:i0 + NI]).astype(np.float32),
            maskq=np.ascontiguousarray(mask[i0:i0 + NI]).astype(np.uint8),
            **shared))
    return in_maps


def split_sync_waits(nc, limit=1):
    """Walrus (this container's neuronxcc) rejects instructions carrying more
    than `limit` sem waits. Hoist excess waits onto per-engine carrier drains
    inserted just before the offending instruction."""
    n_split = 0
    for f in nc.m.functions:
        for bb in f.blocks:
            out = []
            for inst in bb.instructions:
                si = inst.sync_info
                waits = list(si.on_wait) if si and si.on_wait else []
                if len(waits) > limit:
                    extra, keep = waits[:-limit], waits[-limit:]
                    for ci in range(0, len(extra), limit):
                        chunk = extra[ci:ci+limit]
                        nd = mybir.InstDrain(name=f"{inst.name}-wsplit{ci}", ins=[], outs=[])
                        nd.engine = inst.engine
                        nd.sync_info = mybir.SyncInfo(on_wait=chunk, on_update=[])
                        out.append(nd)
                        n_split += 1
                    si.on_wait = keep
                out.append(inst)
            bb.instructions = out
    return n_split


_CACHED = {}


def kernel(**inputs):
    """Full-input entry point: shards over 8 NeuronCores, returns full output."""
    NC_CORES = 8
    NI = N // NC_CORES
    from concourse.bass_utils import run_bass_kernel_spmd

    in_maps = host_prep(inputs, NI=NI, n_cores=NC_CORES)
    if "nc" not in _CACHED:
        nc = build_nc(NI=NI, n_cores=NC_CORES)
        split_sync_waits(nc)
        _CACHED["nc"] = nc
    res = run_bass_kernel_spmd(_CACHED["nc"], in_maps, list(range(NC_CORES)))
    y = np.concatenate([res.results[c]["y"] for c in range(NC_CORES)], axis=0)
    return y[None].astype(np.float32)


# revision 27
# speedup vs baseline: 1.0023x; 1.0023x over previous
"""NodeAttention Trainium2 kernel (per-core program, SPMD over 8 cores).

v2 strategy (per core, i-block of NI=96 query rows; j on partitions):
- pair data arrives host-transposed+cast: PT [C, NJB, NI, 128] bf16 so the
  per-jb DMA is one contiguous 24.5KB run per partition (line-rate HBM).
- LN+bias projection folded into one [128 -> 9] matmul per i with the
  mean-correction folded into the weights host-side:
    W'_ch = lnw_c*wb_ch - s_h/C  (s_h = sum_c lnw_c*wb_ch), col 8 = 1/C.
  The per-head additive constant t_h is dropped (softmax-invariant).
  logits = sim + r * dots',  r = rsqrt(meansq - mu^2 + eps).
- sumsq via DVE square (bf16 2x) + per-i ones-matmul; dots/ss matmuls write
  [j, i, {dots|ss}] PSUM directly (FWL-eligible 128-col bf16 stationaries).
- softmax without max-subtraction; normalizer via ones column in V;
  attention accumulates in PSUM across all 6 j-blocks.
"""
import numpy as np
from contextlib import ExitStack

import concourse.bass as bass
import concourse.tile as tile
from concourse import mybir
from concourse.masks import make_identity

f32 = mybir.dt.float32
bf16 = mybir.dt.bfloat16
u8 = mybir.dt.uint8

N = 768          # sequence length (j axis, also full i)
C = 128          # pair channels
H = 8            # heads
D = 32           # head dim
INNER = 256      # H*D
ND = 256         # node dim
NJB = N // 128   # 6 j-blocks
EPS = 1e-5
CH = 48          # i-half chunk (per-jb dots PSUM bank = 48*10*4 = 1920B)


def _bcast_h(ap2d: bass.AP, h: int) -> bass.AP:
    """[P, F] -> [P, h, F] with step-0 broadcast over the middle dim."""
    ap = list(ap2d.ap)
    assert len(ap) == 2
    return bass.AP(ap2d.tensor, ap2d.offset, [ap[0], [0, h], ap[1]])


def _view(ap_t: bass.AP, frees, off=0) -> bass.AP:
    """Rebuild an AP keeping partition dim, with explicit free [step, num]s."""
    ap = list(ap_t.ap)
    return bass.AP(ap_t.tensor, ap_t.offset + off,
                   [ap[0]] + [list(f) for f in frees])


def build_nc(NI=96, n_cores=8, upto='full'):
    nc = bass.Bass("TRN2", target_bir_lowering=False, debug=False,
                   num_devices=n_cores)
    # pair, host-transposed: PT[c, jb, i, jj] = pair[i, jb*128+jj, c]
    pairt = nc.dram_tensor("pairt", [C, NJB, NI, 128], bf16,
                           kind="ExternalInput").ap()
    node = nc.dram_tensor("node", [N, ND], f32, kind="ExternalInput").ap()
    nodeq = nc.dram_tensor("nodeq", [NI, ND], f32, kind="ExternalInput").ap()
    maskq = nc.dram_tensor("maskq", [NI, N], u8, kind="ExternalInput").ap()
    # wext cols 0-7: lnw*wb - s/C (mean-fold), col 8: 1/C (mean for var)
    wext = nc.dram_tensor("wext", [C, 9], bf16, kind="ExternalInput").ap()
    # wnode cols: [Wq*scale | Wk | Wv | Wg]
    wnode = nc.dram_tensor("wnode", [ND, 4 * INNER], bf16, kind="ExternalInput").ap()
    wout = nc.dram_tensor("wout", [INNER, ND], bf16, kind="ExternalInput").ap()
    lnw = nc.dram_tensor("lnw", [1, ND], f32, kind="ExternalInput").ap()
    lnb = nc.dram_tensor("lnb", [1, ND], f32, kind="ExternalInput").ap()
    bg = nc.dram_tensor("bg", [1, INNER], f32, kind="ExternalInput").ap()
    bout = nc.dram_tensor("bout", [1, ND], f32, kind="ExternalInput").ap()
    y_out = nc.dram_tensor("y", [NI, ND], f32, kind="ExternalOutput").ap()
    dbg = nc.dram_tensor("dbg", [128, 4096], f32, kind="ExternalOutput").ap() \
        if upto == 'dbg' else None

    with tile.TileContext(nc) as tc, ExitStack() as ctx:
        const = ctx.enter_context(tc.tile_pool(name="const", bufs=1))
        persist = ctx.enter_context(tc.tile_pool(name="persist", bufs=1))
        tpp = ctx.enter_context(tc.tile_pool(name="tp", bufs=3))

        # ---- TP0 first (jb0 critical path), then small loads, then TP1-5 ----
        TP_tiles = []
        TP0 = tpp.tile([C, NI, 128], bf16, tag="TP", name="TP0")
        nc.sync.dma_start(out=TP0[:], in_=pairt[:, 0])
        TP_tiles.append(TP0)

        # ---- constants ----
        wext_sb = const.tile([C, 9], bf16)
        nc.scalar.dma_start(out=wext_sb[:], in_=wext)
        ones_sb = const.tile([C, 1], bf16)
        nc.vector.memset(ones_sb[:], 1.0)
        ident = const.tile([128, 128], f32)
        make_identity(nc, ident[:])
        ident_bf = const.tile([128, 128], bf16)
        make_identity(nc, ident_bf[:])
        eps_sb = const.tile([128, 1], f32)
        nc.vector.memset(eps_sb[:], EPS)

        def bload(name, src, cols, dtype=f32):
            t = const.tile([128, cols], dtype, name=name)
            src_b = bass.AP(src.tensor, src.offset, [[0, 128]] + list(src.ap)[1:])
            nc.gpsimd.dma_start(out=t[:], in_=src_b)
            return t
        lnw_sb = bload("lnw_sb", lnw, ND)
        lnb_sb = bload("lnb_sb", lnb, ND)
        bg_sb = bload("bg_sb", bg, INNER)
        bout_sb = bload("bout_sb", bout, ND)
        # node-side weights, feat-major tiles [feat%128, feat//128, cols]
        # (tiles here; DMAs issued in the preamble so TP0 gets bandwidth first)
        wn_sb = const.tile([128, 2, 4 * INNER], bf16)
        wout_sb = const.tile([128, 2, ND], bf16)

        for jb in range(1, NJB):
            TP = tpp.tile([C, NI, 128], bf16, tag="TP", name=f"TP{jb}")
            nc.sync.dma_start(out=TP[:], in_=pairt[:, jb])
            TP_tiles.append(TP)

        # ---- persistent node-derived tensors ----
        kT_sb = persist.tile([32, H, N], bf16)        # k^T [d, h, j]
        qT_sb = persist.tile([32, H, NI], bf16)       # q^T [d, h, i]
        Vx_sb = persist.tile([128, NJB, H, D + 1], bf16)  # v in [j, jb, h, d|1]
        m01T_sb = persist.tile([128, NJB, NI], bf16)  # mask^T in [j, jb, i]
        sig_sb = persist.tile([NI, INNER], f32)       # sigmoid(g) [i, inner]

        # ---- jb-loop pools (open before preamble so jb0 can start early) --
        NH = NI // CH  # i-half count (2)
        sqp = ctx.enter_context(tc.tile_pool(name="tpsq", bufs=3))
        stp = ctx.enter_context(tc.tile_pool(name="stat", bufs=2))
        lgp = ctx.enter_context(tc.tile_pool(name="logit", bufs=2))
        epp = ctx.enter_context(tc.tile_pool(name="epool", bufs=2))
        accp = ctx.enter_context(tc.tile_pool(name="att_acc", bufs=1))
        dps = ctx.enter_context(tc.tile_pool(name="dots_ps", bufs=2, space="PSUM"))

        att_acc = accp.tile([NI, H, D + 1], f32)
        nc.vector.memset(att_acc[:], 0.0)

        def pair_phase(jb):
            """squares, dots', sumsq, stats, u -> logits[jb] (no node deps)."""
            TP = TP_tiles[jb]
            logits = lgp.tile([128, H, NI], f32, tag="logits", name=f"lg{jb}")
            var = stp.tile([128, NH, CH], f32, tag="var")
            Tsqs, dts = [], []
            for ih in range(NH):
                i0 = ih * CH
                TPh = TP[:, i0:i0 + CH, :]
                Tsq = sqp.tile([C, CH, 128], bf16, tag="sq", name=f"sq{jb}_{ih}")
                # split squares across DVE and ACT
                if ih == 0:
                    nc.vector.tensor_mul(Tsq[:], TPh, TPh)
                else:
                    nc.scalar.activation(Tsq[:], TPh,
                                         mybir.ActivationFunctionType.Square)
                Tsqs.append(Tsq)
            for ih in range(NH):
                dt = dps.tile([128, CH, 10], f32, tag=f"dt{ih}", name=f"dt{jb}_{ih}")
                dts.append(dt)
                for il in range(CH):
                    nc.tensor.matmul(dt[:, il, 0:9],
                                     lhsT=TP[:, ih * CH + il, :],
                                     rhs=wext_sb[:])
            for ih in range(NH):
                for il in range(CH):
                    nc.tensor.matmul(dts[ih][:, il, 9:10],
                                     lhsT=Tsqs[ih][:, il, :],
                                     rhs=ones_sb[:])
            for ih in range(NH):
                dt = dts[ih]
                mu_sb = stp.tile([128, CH], f32, tag=f"mu{ih}")
                nc.vector.tensor_copy(mu_sb[:], dt[:, :, 8])
                m2 = stp.tile([128, CH], f32, tag=f"m2{ih}")
                nc.vector.tensor_mul(m2[:], mu_sb[:], mu_sb[:])
                nc.vector.scalar_tensor_tensor(
                    out=var[:, ih, :], in0=dt[:, :, 9], scalar=1.0 / C, in1=m2[:],
                    op0=mybir.AluOpType.mult, op1=mybir.AluOpType.subtract)
            sd = stp.tile([128, NH, CH], f32, tag="sd")
            nc.scalar.activation(sd[:], var[:],
                                 mybir.ActivationFunctionType.Sqrt,
                                 bias=eps_sb[:])
            r = stp.tile([128, NH, CH], f32, tag="r")
            nc.vector.reciprocal(r[:], sd[:])
            for ih in range(NH):
                i0 = ih * CH
                dots_v = _view(dts[ih], [[1, H], [10, CH]])
                r_b = _view(r[:, ih, :], [[0, H], [1, CH]])
                nc.vector.tensor_mul(logits[:, :, i0:i0 + CH], dots_v, r_b)
            return logits

        logits_t = {0: pair_phase(0)}

        # ================= node preamble (overlaps jb0 pair phase) ==========
        with tc.tile_pool(name="nodep", bufs=1) as npool, \
             tc.tile_pool(name="node_ps", bufs=1, space="PSUM") as nps:
            x_all = npool.tile([128, N // 128, ND], f32)
            stats = npool.tile([128, 6], f32)
            mv = npool.tile([128, 2], f32)
            sd = npool.tile([128, 1], f32)
            rln = npool.tile([128, 1], f32)

            def layernorm_tile(xt, nrows):
                nc.vector.bn_stats(out=stats[:nrows, :], in_=xt)
                nc.vector.bn_aggr(out=mv[:nrows, :], in_=stats[:nrows, :])
                nc.scalar.activation(sd[:nrows, :], mv[:nrows, 1:2],
                                     mybir.ActivationFunctionType.Sqrt,
                                     bias=eps_sb[:nrows, :])
                nc.vector.reciprocal(rln[:nrows, :], sd[:nrows, :])
                nc.vector.tensor_scalar(out=xt, in0=xt,
                                        scalar1=mv[:nrows, 0:1],
                                        scalar2=rln[:nrows, :],
                                        op0=mybir.AluOpType.subtract,
                                        op1=mybir.AluOpType.mult)
                nc.vector.tensor_mul(xt, xt, lnw_sb[:nrows, :])
                nc.vector.tensor_add(xt, xt, lnb_sb[:nrows, :])

            for t in range(N // 128):
                nc.scalar.dma_start(out=x_all[:, t, :], in_=node[t * 128:(t + 1) * 128, :])
                layernorm_tile(x_all[:, t, :], 128)

            xq = npool.tile([NI, ND], f32)
            nc.scalar.dma_start(out=xq[:], in_=nodeq)
            nc.scalar.dma_start(out=wn_sb[:],
                                in_=wnode.rearrange("(kt p) c -> p kt c", p=128))
            nc.scalar.dma_start(out=wout_sb[:],
                                in_=wout.rearrange("(kt p) c -> p kt c", p=128))
            layernorm_tile(xq[:], NI)

            # -- transposes: xT [feat, j], xqT [feat, i] --
            xT_sb = npool.tile([128, 2, N], bf16)
            xqT_sb = npool.tile([128, 2, NI], bf16)
            for t in range(N // 128):
                for kt in range(2):
                    tp = nps.tile([128, 128], f32, tag="xpose")
                    nc.tensor.transpose(tp[:], x_all[:, t, kt * 128:(kt + 1) * 128], ident[:])
                    nc.vector.tensor_copy(xT_sb[:, kt, t * 128:(t + 1) * 128], tp[:])
            for kt in range(2):
                tp = nps.tile([128, NI], f32, tag="xpose")
                nc.tensor.transpose(tp[:], xq[:, kt * 128:(kt + 1) * 128], ident[:NI, :NI])
                nc.vector.tensor_copy(xqT_sb[:, kt, :], tp[:])

            # -- k^T = Wk^T @ x^T : per-head [d, j] at partition base 0 --
            for h in range(H):
                for n0 in range(0, N, 384):
                    kp = nps.tile([32, 384], f32, tag="nmm")
                    for kt in range(2):
                        nc.tensor.matmul(
                            kp[:],
                            lhsT=wn_sb[:, kt, INNER + h * D:INNER + (h + 1) * D],
                            rhs=xT_sb[:, kt, n0:n0 + 384],
                            start=(kt == 0), stop=(kt == 1))
                    nc.vector.tensor_copy(kT_sb[:, h, n0:n0 + 384], kp[:])

            # -- v = x @ Wv -> Vx [j, jb, h, d] + ones column --
            for jb in range(NJB):
                vp = nps.tile([128, INNER], f32, tag="vmm")
                for kt in range(2):
                    nc.tensor.matmul(vp[:], lhsT=xT_sb[:, kt, jb * 128:(jb + 1) * 128],
                                     rhs=wn_sb[:, kt, 2 * INNER:3 * INNER],
                                     start=(kt == 0), stop=(kt == 1))
                nc.vector.tensor_copy(Vx_sb[:, jb, :, 0:D],
                                      vp[:].rearrange("p (h d) -> p h d", h=H))
            nc.vector.memset(Vx_sb[:, :, :, D:D + 1], 1.0)

            # -- q^T = (Wq*scale)^T @ xq^T : per-head [d, i] --
            for h in range(H):
                qp = nps.tile([32, NI], f32, tag="nmm")
                for kt in range(2):
                    nc.tensor.matmul(qp[:], lhsT=wn_sb[:, kt, h * D:(h + 1) * D],
                                     rhs=xqT_sb[:, kt, :],
                                     start=(kt == 0), stop=(kt == 1))
                nc.vector.tensor_copy(qT_sb[:, h, :], qp[:])

            # -- g = xq @ Wg + bg ; sig = sigmoid(g) --
            gp = nps.tile([NI, INNER], f32, tag="gmm")
            for kt in range(2):
                nc.tensor.matmul(gp[:], lhsT=xqT_sb[:, kt, :],
                                 rhs=wn_sb[:, kt, 3 * INNER:4 * INNER],
                                 start=(kt == 0), stop=(kt == 1))
            gt = npool.tile([NI, INNER], f32)
            nc.vector.tensor_add(gt[:], gp[:], bg_sb[:NI, :])
            nc.scalar.activation(sig_sb[:NI, :], gt[:],
                                 mybir.ActivationFunctionType.Sigmoid)

            # -- mask: [i, j] u8 -> m01T [j, jb, i] bf16 --
            mu8 = npool.tile([NI, N], u8)
            nc.scalar.dma_start(out=mu8[:], in_=maskq)
            mbf = npool.tile([NI, N], bf16)
            nc.vector.tensor_copy(mbf[:], mu8[:])
            for jb in range(NJB):
                tp = nps.tile([128, NI], bf16, tag="xpose")
                nc.tensor.transpose(tp[:], mbf[:, jb * 128:(jb + 1) * 128],
                                    ident_bf[:NI, :NI])
                nc.vector.tensor_copy(m01T_sb[:, jb, :], tp[:])

        # ============ pipelined jb loop: att/mask delayed one block =========
        E_t, attps_t = {}, {}
        with tc.tile_pool(name="sim_ps", bufs=1, space="PSUM") as simp, \
             tc.tile_pool(name="att_ps", bufs=2, space="PSUM") as attp:
            for k in range(NJB + 1):
                # exp one iteration late: keeps ACT queue stall-free
                if 1 <= k:
                    j = k - 1
                    E = epp.tile([128, H, NI], bf16, tag="E", name=f"E{j}")
                    nc.scalar.activation(E[:], logits_t.pop(j),
                                         mybir.ActivationFunctionType.Exp)
                    E_t[j] = E
                if 1 <= k <= NJB - 1:
                    logits_t[k] = pair_phase(k)
                if k <= NJB - 1:
                    sim_ps = simp.tile([128, H, 128], f32, tag="sim", name=f"sim{k}")
                    for h in range(H):
                        nc.tensor.matmul(
                            sim_ps[:, h, :NI],
                            lhsT=kT_sb[:, h, k * 128:(k + 1) * 128],
                            rhs=qT_sb[:, h, :])
                    lg = logits_t[k]
                    nc.vector.tensor_add(lg[:], lg[:], sim_ps[:, :, :NI])
                if k >= 1:
                    j = k - 1
                    E = E_t.pop(j)
                    nc.gpsimd.tensor_mul(E[:], E[:], _bcast_h(m01T_sb[:, j, :], H))
                    att_ps = attp.tile([NI, H, D + 1], f32, tag="attjb",
                                       name=f"att{j}")
                    for h in range(H):
                        nc.tensor.matmul(att_ps[:, h, :], lhsT=E[:, h, :],
                                         rhs=Vx_sb[:, j, h, :])
                    attps_t[j] = att_ps
                if k >= 2:
                    ap = attps_t.pop(k - 2)
                    nc.vector.tensor_add(att_acc[:], att_acc[:], ap[:])
            ap = attps_t.pop(NJB - 1)
            nc.vector.tensor_add(att_acc[:], att_acc[:], ap[:])

        # ---- finalize ----
        with tc.tile_pool(name="fin", bufs=1) as fin, \
             tc.tile_pool(name="fin_ps", bufs=2, space="PSUM") as finp:
            den_r = fin.tile([NI, H], f32)
            den_v = _view(att_acc, [[D + 1, H]], off=D)
            nc.vector.reciprocal(den_r[:], den_v)
            att_f = fin.tile([NI, INNER], f32)
            num_v = _view(att_acc, [[D + 1, H], [1, D]])
            den_b = _view(den_r, [[1, H], [0, D]])
            nc.vector.tensor_mul(att_f[:].rearrange("p (h d) -> p h d", h=H),
                                 num_v, den_b)
            gated = fin.tile([NI, INNER], f32)
            nc.vector.tensor_mul(gated[:], att_f[:], sig_sb[:NI, :])

            gT_sb = fin.tile([128, 2, NI], bf16)
            for kt in range(2):
                tp = finp.tile([128, NI], f32, tag="gpose")
                nc.tensor.transpose(tp[:], gated[:, kt * 128:(kt + 1) * 128],
                                    ident[:NI, :NI])
                nc.vector.tensor_copy(gT_sb[:, kt, :], tp[:])

            y_ps = finp.tile([NI, ND], f32, tag="ymm")
            for kt in range(2):
                nc.tensor.matmul(y_ps[:], lhsT=gT_sb[:, kt, :],
                                 rhs=wout_sb[:, kt, :],
                                 start=(kt == 0), stop=(kt == 1))
            y_sb = fin.tile([NI, ND], f32)
            nc.vector.tensor_add(y_sb[:], y_ps[:], bout_sb[:NI, :])
            nc.scalar.dma_start(out=y_out, in_=y_sb[:])

    return nc


def host_prep(inputs, NI=96, n_cores=8):
    """Slice/fold/transpose FULL inputs into per-core in_maps."""
    import ml_dtypes
    node_feats = np.asarray(inputs["node_feats"])[0]      # [N, ND]
    pair_feats = np.asarray(inputs["pair_feats"])[0]      # [N, N, C]
    mask = np.asarray(inputs["mask"])[0]                  # [N, N] bool
    lnw = np.asarray(inputs["ln_node_w"]).reshape(1, ND)
    lnb = np.asarray(inputs["ln_node_b"]).reshape(1, ND)
    lpw = np.asarray(inputs["ln_pair_w"])                 # [C]
    lpb = np.asarray(inputs["ln_pair_b"])                 # [C]  (t_h drops out)
    w_qkv = np.asarray(inputs["w_qkv"])                   # [ND, 3*INNER]
    w_g = np.asarray(inputs["w_g"])                       # [ND, INNER]
    b_g = np.asarray(inputs["b_g"]).reshape(1, INNER)
    w_bias = np.asarray(inputs["w_bias"])                 # [C, H]
    w_out = np.asarray(inputs["w_out"])                   # [INNER, ND]
    b_out = np.asarray(inputs["b_out"]).reshape(1, ND)

    Wp = (lpw[:, None] * w_bias).astype(np.float32)       # [C, H]
    s_h = Wp.sum(0)                                       # [H]
    wext = np.zeros((C, 9), np.float32)
    wext[:, 0:H] = Wp - s_h[None, :] / C                  # mean-fold
    wext[:, 8] = 1.0 / C
    wext = wext.astype(ml_dtypes.bfloat16)

    scale = D ** -0.5
    wnode = np.concatenate([w_qkv[:, 0:INNER] * scale,
                            w_qkv[:, INNER:2 * INNER],
                            w_qkv[:, 2 * INNER:3 * INNER],
                            w_g], axis=1).astype(ml_dtypes.bfloat16)
    woutb = w_out.astype(ml_dtypes.bfloat16)

    shared = dict(node=node_feats.astype(np.float32), wext=wext, wnode=wnode,
                  wout=woutb, lnw=lnw.astype(np.float32), lnb=lnb.astype(np.float32),
                  bg=b_g.astype(np.float32), bout=b_out.astype(np.float32))
    in_maps = []
    for c in range(n_cores):
        i0 = c * NI
        q = pair_feats[i0:i0 + NI].transpose(2, 0, 1)     # [C, NI, N]
        pt = q.reshape(C, NI, NJB, 128).transpose(0, 2, 1, 3)  # [C, jb, i, jj]
        in_maps.append(dict(
            pairt=np.ascontiguousarray(pt).astype(ml_dtypes.bfloat16),
            nodeq=np.ascontiguousarray(node_feats[i0:i0 + NI]).astype(np.float32),
            maskq=np.ascontiguousarray(mask[i0:i0 + NI]).astype(np.uint8),
            **shared))
    return in_maps


def split_sync_waits(nc, limit=1):
    """Walrus (this container's neuronxcc) rejects instructions carrying more
    than `limit` sem waits. Hoist excess waits onto per-engine carrier drains
    inserted just before the offending instruction."""
    n_split = 0
    for f in nc.m.functions:
        for bb in f.blocks:
            out = []
            for inst in bb.instructions:
                si = inst.sync_info
                waits = list(si.on_wait) if si and si.on_wait else []
                if len(waits) > limit:
                    extra, keep = waits[:-limit], waits[-limit:]
                    for ci in range(0, len(extra), limit):
                        chunk = extra[ci:ci+limit]
                        nd = mybir.InstDrain(name=f"{inst.name}-wsplit{ci}", ins=[], outs=[])
                        nd.engine = inst.engine
                        nd.sync_info = mybir.SyncInfo(on_wait=chunk, on_update=[])
                        out.append(nd)
                        n_split += 1
                    si.on_wait = keep
                out.append(inst)
            bb.instructions = out
    return n_split


_CACHED = {}


def kernel(**inputs):
    """Full-input entry point: shards over 8 NeuronCores, returns full output."""
    NC_CORES = 8
    NI = N // NC_CORES
    from concourse.bass_utils import run_bass_kernel_spmd

    in_maps = host_prep(inputs, NI=NI, n_cores=NC_CORES)
    if "nc" not in _CACHED:
        nc = build_nc(NI=NI, n_cores=NC_CORES)
        split_sync_waits(nc)
        _CACHED["nc"] = nc
    res = run_bass_kernel_spmd(_CACHED["nc"], in_maps, list(range(NC_CORES)))
    y = np.concatenate([res.results[c]["y"] for c in range(NC_CORES)], axis=0)
    return y[None].astype(np.float32)


# revision 32
# speedup vs baseline: 1.1246x; 1.1220x over previous
"""NodeAttention Trainium2 kernel (per-core program, SPMD over 8 cores).

v2 strategy (per core, i-block of NI=96 query rows; j on partitions):
- pair data arrives host-transposed+cast: PT [C, NJB, NI, 128] bf16 so the
  per-jb DMA is one contiguous 24.5KB run per partition (line-rate HBM).
- LN+bias projection folded into one [128 -> 9] matmul per i with the
  mean-correction folded into the weights host-side:
    W'_ch = lnw_c*wb_ch - s_h/C  (s_h = sum_c lnw_c*wb_ch), col 8 = 1/C.
  The per-head additive constant t_h is dropped (softmax-invariant).
  logits = sim + r * dots',  r = rsqrt(meansq - mu^2 + eps).
- sumsq via DVE square (bf16 2x) + per-i ones-matmul; dots/ss matmuls write
  [j, i, {dots|ss}] PSUM directly (FWL-eligible 128-col bf16 stationaries).
- softmax without max-subtraction; normalizer via ones column in V;
  attention accumulates in PSUM across all 6 j-blocks.
"""
import numpy as np
from contextlib import ExitStack

import concourse.bass as bass
import concourse.tile as tile
from concourse import mybir
from concourse.masks import make_identity

f32 = mybir.dt.float32
bf16 = mybir.dt.bfloat16
u8 = mybir.dt.uint8

N = 768          # sequence length (j axis, also full i)
C = 128          # pair channels
H = 8            # heads
D = 32           # head dim
INNER = 256      # H*D
ND = 256         # node dim
NJB = N // 128   # 6 j-blocks
EPS = 1e-5
CH = 48          # i-half chunk (per-jb dots PSUM bank = 48*10*4 = 1920B)


def _bcast_h(ap2d: bass.AP, h: int) -> bass.AP:
    """[P, F] -> [P, h, F] with step-0 broadcast over the middle dim."""
    ap = list(ap2d.ap)
    assert len(ap) == 2
    return bass.AP(ap2d.tensor, ap2d.offset, [ap[0], [0, h], ap[1]])


def _view(ap_t: bass.AP, frees, off=0) -> bass.AP:
    """Rebuild an AP keeping partition dim, with explicit free [step, num]s."""
    ap = list(ap_t.ap)
    return bass.AP(ap_t.tensor, ap_t.offset + off,
                   [ap[0]] + [list(f) for f in frees])


def build_nc(NI=96, n_cores=8, upto='full'):
    nc = bass.Bass("TRN2", target_bir_lowering=False, debug=False,
                   num_devices=n_cores)
    # pair, host-transposed: PT[c, jb, i, jj] = pair[i, jb*128+jj, c]
    pairt = nc.dram_tensor("pairt", [C, NJB, NI, 128], bf16,
                           kind="ExternalInput").ap()
    node = nc.dram_tensor("node", [N, ND], f32, kind="ExternalInput").ap()
    nodeq = nc.dram_tensor("nodeq", [NI, ND], f32, kind="ExternalInput").ap()
    maskq = nc.dram_tensor("maskq", [NI, N], u8, kind="ExternalInput").ap()
    # wext cols 0-7: lnw*wb - s/C (mean-fold), col 8: 1/C (mean for var)
    wext = nc.dram_tensor("wext", [C, 9], bf16, kind="ExternalInput").ap()
    # wnode cols: [Wq*scale | Wk | Wv | Wg]
    wnode = nc.dram_tensor("wnode", [ND, 4 * INNER], bf16, kind="ExternalInput").ap()
    wout = nc.dram_tensor("wout", [INNER, ND], bf16, kind="ExternalInput").ap()
    lnw = nc.dram_tensor("lnw", [1, ND], f32, kind="ExternalInput").ap()
    lnb = nc.dram_tensor("lnb", [1, ND], f32, kind="ExternalInput").ap()
    bg = nc.dram_tensor("bg", [1, INNER], f32, kind="ExternalInput").ap()
    bout = nc.dram_tensor("bout", [1, ND], f32, kind="ExternalInput").ap()
    y_out = nc.dram_tensor("y", [NI, ND], f32, kind="ExternalOutput").ap()
    dbg = nc.dram_tensor("dbg", [128, 4096], f32, kind="ExternalOutput").ap() \
        if upto == 'dbg' else None

    with tile.TileContext(nc) as tc, ExitStack() as ctx:
        const = ctx.enter_context(tc.tile_pool(name="const", bufs=1))
        persist = ctx.enter_context(tc.tile_pool(name="persist", bufs=1))
        tpp = ctx.enter_context(tc.tile_pool(name="tp", bufs=3))

        # ---- all input DMAs share ONE queue (sync), ordered by need:
        # wext, TP0, node, wn, TP1, nodeq/wout/mask, TP2..TP5.  Two queues
        # starve each other (SDMA round-robins per packet, and the pair
        # packets are 24.5KB vs ~1KB) -- so order explicitly on one queue.
        wext_sb = const.tile([C, 9], bf16)
        nc.sync.dma_start(out=wext_sb[:], in_=wext)
        TP_tiles = []
        for jb in range(NJB):
            TP_tiles.append(tpp.tile([C, NI, 128], bf16, tag="TP", name=f"TP{jb}"))
        nc.sync.dma_start(out=TP_tiles[0][:], in_=pairt[:, 0])
        ones_sb = const.tile([C, 1], bf16)
        nc.vector.memset(ones_sb[:], 1.0)
        ident = const.tile([128, 128], f32)
        make_identity(nc, ident[:])
        ident_bf = const.tile([128, 128], bf16)
        make_identity(nc, ident_bf[:])
        eps_sb = const.tile([128, 1], f32)
        nc.vector.memset(eps_sb[:], EPS)

        def bload(name, src, cols, dtype=f32):
            t = const.tile([128, cols], dtype, name=name)
            src_b = bass.AP(src.tensor, src.offset, [[0, 128]] + list(src.ap)[1:])
            nc.gpsimd.dma_start(out=t[:], in_=src_b)
            return t
        lnw_sb = bload("lnw_sb", lnw, ND)
        lnb_sb = bload("lnb_sb", lnb, ND)
        bg_sb = bload("bg_sb", bg, INNER)
        bout_sb = bload("bout_sb", bout, ND)
        # node-side weights, feat-major tiles [feat%128, feat//128, cols]
        # (tiles here; DMAs issued in the preamble in need-order)
        wn_sb = const.tile([128, 2, 4 * INNER], bf16)
        wout_sb = const.tile([128, 2, ND], bf16)

        # ---- persistent node-derived tensors ----
        kT_sb = persist.tile([32, H, N], bf16)        # k^T [d, h, j]
        qT_sb = persist.tile([32, H, NI], bf16)       # q^T [d, h, i]
        Vx_sb = persist.tile([128, NJB, H, D + 1], bf16)  # v in [j, jb, h, d|1]
        m01T_sb = persist.tile([128, NJB, NI], bf16)  # mask^T in [j, jb, i]
        sig_sb = persist.tile([NI, INNER], f32)       # sigmoid(g) [i, inner]

        # ---- jb-loop pools (open before preamble so jb0 can start early) --
        NH = NI // CH  # i-half count (2)
        sqp = ctx.enter_context(tc.tile_pool(name="tpsq", bufs=3))
        stp = ctx.enter_context(tc.tile_pool(name="stat", bufs=2))
        lgp = ctx.enter_context(tc.tile_pool(name="logit", bufs=2))
        epp = ctx.enter_context(tc.tile_pool(name="epool", bufs=2))
        accp = ctx.enter_context(tc.tile_pool(name="att_acc", bufs=1))
        dps = ctx.enter_context(tc.tile_pool(name="dots_ps", bufs=2, space="PSUM"))

        att_acc = accp.tile([NI, H, D + 1], f32)
        nc.vector.memset(att_acc[:], 0.0)

        def pair_phase(jb):
            """squares, dots', sumsq, stats, u -> logits[jb] (no node deps)."""
            TP = TP_tiles[jb]
            logits = lgp.tile([128, H, NI], f32, tag="logits", name=f"lg{jb}")
            var = stp.tile([128, NH, CH], f32, tag="var")
            Tsqs, dts = [], []
            for ih in range(NH):
                i0 = ih * CH
                TPh = TP[:, i0:i0 + CH, :]
                Tsq = sqp.tile([C, CH, 128], bf16, tag="sq", name=f"sq{jb}_{ih}")
                # squares: 2/3 DVE, 1/3 ACT (ACT Square is Accel=1)
                if ih == 0:
                    nc.vector.tensor_mul(Tsq[:], TPh, TPh)
                else:
                    nc.vector.tensor_mul(Tsq[:, 0:16, :], TP[:, i0:i0 + 16, :],
                                         TP[:, i0:i0 + 16, :])
                    nc.scalar.activation(Tsq[:, 16:CH, :],
                                         TP[:, i0 + 16:i0 + CH, :],
                                         mybir.ActivationFunctionType.Square)
                Tsqs.append(Tsq)
            for ih in range(NH):
                dt = dps.tile([128, CH, 10], f32, tag=f"dt{ih}", name=f"dt{jb}_{ih}")
                dts.append(dt)
                for il in range(CH):
                    nc.tensor.matmul(dt[:, il, 0:9],
                                     lhsT=TP[:, ih * CH + il, :],
                                     rhs=wext_sb[:])
            for ih in range(NH):
                for il in range(CH):
                    nc.tensor.matmul(dts[ih][:, il, 9:10],
                                     lhsT=Tsqs[ih][:, il, :],
                                     rhs=ones_sb[:])
            for ih in range(NH):
                dt = dts[ih]
                mu_sb = stp.tile([128, CH], f32, tag=f"mu{ih}")
                nc.vector.tensor_copy(mu_sb[:], dt[:, :, 8])
                m2 = stp.tile([128, CH], f32, tag=f"m2{ih}")
                nc.vector.tensor_mul(m2[:], mu_sb[:], mu_sb[:])
                nc.vector.scalar_tensor_tensor(
                    out=var[:, ih, :], in0=dt[:, :, 9], scalar=1.0 / C, in1=m2[:],
                    op0=mybir.AluOpType.mult, op1=mybir.AluOpType.subtract)
            sd = stp.tile([128, NH, CH], f32, tag="sd")
            nc.scalar.activation(sd[:], var[:],
                                 mybir.ActivationFunctionType.Sqrt,
                                 bias=eps_sb[:])
            r = stp.tile([128, NH, CH], f32, tag="r")
            nc.vector.reciprocal(r[:], sd[:])
            for ih in range(NH):
                i0 = ih * CH
                dots_v = _view(dts[ih], [[1, H], [10, CH]])
                r_b = _view(r[:, ih, :], [[0, H], [1, CH]])
                nc.vector.tensor_mul(logits[:, :, i0:i0 + CH], dots_v, r_b)
            return logits

        logits_t = {0: pair_phase(0)}

        # ================= node preamble (overlaps jb0 pair phase) ==========
        with tc.tile_pool(name="nodep", bufs=1) as npool, \
             tc.tile_pool(name="node_ps", bufs=1, space="PSUM") as nps:
            x_all = npool.tile([128, N // 128, ND], f32)
            stats = npool.tile([128, 6], f32)
            mv = npool.tile([128, 2], f32)
            sd = npool.tile([128, 1], f32)
            rln = npool.tile([128, 1], f32)

            def layernorm_tile(xt, nrows):
                nc.vector.bn_stats(out=stats[:nrows, :], in_=xt)
                nc.vector.bn_aggr(out=mv[:nrows, :], in_=stats[:nrows, :])
                nc.scalar.activation(sd[:nrows, :], mv[:nrows, 1:2],
                                     mybir.ActivationFunctionType.Sqrt,
                                     bias=eps_sb[:nrows, :])
                nc.vector.reciprocal(rln[:nrows, :], sd[:nrows, :])
                nc.vector.tensor_scalar(out=xt, in0=xt,
                                        scalar1=mv[:nrows, 0:1],
                                        scalar2=rln[:nrows, :],
                                        op0=mybir.AluOpType.subtract,
                                        op1=mybir.AluOpType.mult)
                nc.vector.tensor_mul(xt, xt, lnw_sb[:nrows, :])
                nc.vector.tensor_add(xt, xt, lnb_sb[:nrows, :])

            for t in range(N // 128):
                nc.sync.dma_start(out=x_all[:, t, :], in_=node[t * 128:(t + 1) * 128, :])
                layernorm_tile(x_all[:, t, :], 128)

            # need-ordered on the single input queue
            nc.sync.dma_start(out=TP_tiles[1][:], in_=pairt[:, 1])
            xq = npool.tile([NI, ND], f32)
            mu8 = npool.tile([NI, N], u8)
            nc.sync.dma_start(out=wn_sb[:],
                              in_=wnode.rearrange("(kt p) c -> p kt c", p=128))
            nc.sync.dma_start(out=xq[:], in_=nodeq)
            nc.sync.dma_start(out=wout_sb[:],
                              in_=wout.rearrange("(kt p) c -> p kt c", p=128))
            nc.sync.dma_start(out=mu8[:], in_=maskq)
            for jb in range(2, NJB):
                nc.sync.dma_start(out=TP_tiles[jb][:], in_=pairt[:, jb])
            layernorm_tile(xq[:], NI)

            # -- transposes: xT [feat, j], xqT [feat, i] --
            xT_sb = npool.tile([128, 2, N], bf16)
            xqT_sb = npool.tile([128, 2, NI], bf16)
            for t in range(N // 128):
                for kt in range(2):
                    tp = nps.tile([128, 128], f32, tag="xpose")
                    nc.tensor.transpose(tp[:], x_all[:, t, kt * 128:(kt + 1) * 128], ident[:])
                    nc.vector.tensor_copy(xT_sb[:, kt, t * 128:(t + 1) * 128], tp[:])
            for kt in range(2):
                tp = nps.tile([128, NI], f32, tag="xpose")
                nc.tensor.transpose(tp[:], xq[:, kt * 128:(kt + 1) * 128], ident[:NI, :NI])
                nc.vector.tensor_copy(xqT_sb[:, kt, :], tp[:])

            # -- k^T = Wk^T @ x^T : per-head [d, j] at partition base 0 --
            for h in range(H):
                for n0 in range(0, N, 384):
                    kp = nps.tile([32, 384], f32, tag="nmm")
                    for kt in range(2):
                        nc.tensor.matmul(
                            kp[:],
                            lhsT=wn_sb[:, kt, INNER + h * D:INNER + (h + 1) * D],
                            rhs=xT_sb[:, kt, n0:n0 + 384],
                            start=(kt == 0), stop=(kt == 1))
                    nc.vector.tensor_copy(kT_sb[:, h, n0:n0 + 384], kp[:])

            # -- v = x @ Wv -> Vx [j, jb, h, d] + ones column --
            for jb in range(NJB):
                vp = nps.tile([128, INNER], f32, tag="vmm")
                for kt in range(2):
                    nc.tensor.matmul(vp[:], lhsT=xT_sb[:, kt, jb * 128:(jb + 1) * 128],
                                     rhs=wn_sb[:, kt, 2 * INNER:3 * INNER],
                                     start=(kt == 0), stop=(kt == 1))
                nc.vector.tensor_copy(Vx_sb[:, jb, :, 0:D],
                                      vp[:].rearrange("p (h d) -> p h d", h=H))
            nc.vector.memset(Vx_sb[:, :, :, D:D + 1], 1.0)

            # -- q^T = (Wq*scale)^T @ xq^T : per-head [d, i] --
            for h in range(H):
                qp = nps.tile([32, NI], f32, tag="nmm")
                for kt in range(2):
                    nc.tensor.matmul(qp[:], lhsT=wn_sb[:, kt, h * D:(h + 1) * D],
                                     rhs=xqT_sb[:, kt, :],
                                     start=(kt == 0), stop=(kt == 1))
                nc.vector.tensor_copy(qT_sb[:, h, :], qp[:])

            # -- g = xq @ Wg + bg ; sig = sigmoid(g) --
            gp = nps.tile([NI, INNER], f32, tag="gmm")
            for kt in range(2):
                nc.tensor.matmul(gp[:], lhsT=xqT_sb[:, kt, :],
                                 rhs=wn_sb[:, kt, 3 * INNER:4 * INNER],
                                 start=(kt == 0), stop=(kt == 1))
            gt = npool.tile([NI, INNER], f32)
            nc.vector.tensor_add(gt[:], gp[:], bg_sb[:NI, :])
            nc.scalar.activation(sig_sb[:NI, :], gt[:],
                                 mybir.ActivationFunctionType.Sigmoid)

            # -- mask: [i, j] u8 -> m01T [j, jb, i] bf16 --
            mbf = npool.tile([NI, N], bf16)
            nc.vector.tensor_copy(mbf[:], mu8[:])
            for jb in range(NJB):
                tp = nps.tile([128, NI], bf16, tag="xpose")
                nc.tensor.transpose(tp[:], mbf[:, jb * 128:(jb + 1) * 128],
                                    ident_bf[:NI, :NI])
                nc.vector.tensor_copy(m01T_sb[:, jb, :], tp[:])

        # ============ pipelined jb loop: att/mask delayed one block =========
        E_t, attps_t = {}, {}
        with tc.tile_pool(name="sim_ps", bufs=1, space="PSUM") as simp, \
             tc.tile_pool(name="att_ps", bufs=2, space="PSUM") as attp:
            for k in range(NJB + 1):
                # exp one iteration late: keeps ACT queue stall-free
                if 1 <= k:
                    j = k - 1
                    E = epp.tile([128, H, NI], bf16, tag="E", name=f"E{j}")
                    nc.scalar.activation(E[:], logits_t.pop(j),
                                         mybir.ActivationFunctionType.Exp)
                    E_t[j] = E
                if 1 <= k <= NJB - 1:
                    logits_t[k] = pair_phase(k)
                if k <= NJB - 1:
                    sim_ps = simp.tile([128, H, 128], f32, tag="sim", name=f"sim{k}")
                    for h in range(H):
                        nc.tensor.matmul(
                            sim_ps[:, h, :NI],
                            lhsT=kT_sb[:, h, k * 128:(k + 1) * 128],
                            rhs=qT_sb[:, h, :])
                    lg = logits_t[k]
                    nc.vector.tensor_add(lg[:], lg[:], sim_ps[:, :, :NI])
                if k >= 1:
                    j = k - 1
                    E = E_t.pop(j)
                    nc.gpsimd.tensor_mul(E[:], E[:], _bcast_h(m01T_sb[:, j, :], H))
                    att_ps = attp.tile([NI, H, D + 1], f32, tag="attjb",
                                       name=f"att{j}")
                    for h in range(H):
                        nc.tensor.matmul(att_ps[:, h, :], lhsT=E[:, h, :],
                                         rhs=Vx_sb[:, j, h, :])
                    attps_t[j] = att_ps
                if k >= 2:
                    ap = attps_t.pop(k - 2)
                    nc.vector.tensor_add(att_acc[:], att_acc[:], ap[:])
            ap = attps_t.pop(NJB - 1)
            nc.vector.tensor_add(att_acc[:], att_acc[:], ap[:])

        # ---- finalize ----
        with tc.tile_pool(name="fin", bufs=1) as fin, \
             tc.tile_pool(name="fin_ps", bufs=2, space="PSUM") as finp:
            den_r = fin.tile([NI, H], f32)
            den_v = _view(att_acc, [[D + 1, H]], off=D)
            nc.vector.reciprocal(den_r[:], den_v)
            att_f = fin.tile([NI, INNER], f32)
            num_v = _view(att_acc, [[D + 1, H], [1, D]])
            den_b = _view(den_r, [[1, H], [0, D]])
            nc.vector.tensor_mul(att_f[:].rearrange("p (h d) -> p h d", h=H),
                                 num_v, den_b)
            gated = fin.tile([NI, INNER], f32)
            nc.vector.tensor_mul(gated[:], att_f[:], sig_sb[:NI, :])

            gT_sb = fin.tile([128, 2, NI], bf16)
            for kt in range(2):
                tp = finp.tile([128, NI], f32, tag="gpose")
                nc.tensor.transpose(tp[:], gated[:, kt * 128:(kt + 1) * 128],
                                    ident[:NI, :NI])
                nc.vector.tensor_copy(gT_sb[:, kt, :], tp[:])

            y_ps = finp.tile([NI, ND], f32, tag="ymm")
            for kt in range(2):
                nc.tensor.matmul(y_ps[:], lhsT=gT_sb[:, kt, :],
                                 rhs=wout_sb[:, kt, :],
                                 start=(kt == 0), stop=(kt == 1))
            y_sb = fin.tile([NI, ND], f32)
            nc.vector.tensor_add(y_sb[:], y_ps[:], bout_sb[:NI, :])
            nc.scalar.dma_start(out=y_out, in_=y_sb[:])

    return nc


def host_prep(inputs, NI=96, n_cores=8):
    """Slice/fold/transpose FULL inputs into per-core in_maps."""
    import ml_dtypes
    node_feats = np.asarray(inputs["node_feats"])[0]      # [N, ND]
    pair_feats = np.asarray(inputs["pair_feats"])[0]      # [N, N, C]
    mask = np.asarray(inputs["mask"])[0]                  # [N, N] bool
    lnw = np.asarray(inputs["ln_node_w"]).reshape(1, ND)
    lnb = np.asarray(inputs["ln_node_b"]).reshape(1, ND)
    lpw = np.asarray(inputs["ln_pair_w"])                 # [C]
    lpb = np.asarray(inputs["ln_pair_b"])                 # [C]  (t_h drops out)
    w_qkv = np.asarray(inputs["w_qkv"])                   # [ND, 3*INNER]
    w_g = np.asarray(inputs["w_g"])                       # [ND, INNER]
    b_g = np.asarray(inputs["b_g"]).reshape(1, INNER)
    w_bias = np.asarray(inputs["w_bias"])                 # [C, H]
    w_out = np.asarray(inputs["w_out"])                   # [INNER, ND]
    b_out = np.asarray(inputs["b_out"]).reshape(1, ND)

    Wp = (lpw[:, None] * w_bias).astype(np.float32)       # [C, H]
    s_h = Wp.sum(0)                                       # [H]
    wext = np.zeros((C, 9), np.float32)
    wext[:, 0:H] = Wp - s_h[None, :] / C                  # mean-fold
    wext[:, 8] = 1.0 / C
    wext = wext.astype(ml_dtypes.bfloat16)

    scale = D ** -0.5
    wnode = np.concatenate([w_qkv[:, 0:INNER] * scale,
                            w_qkv[:, INNER:2 * INNER],
                            w_qkv[:, 2 * INNER:3 * INNER],
                            w_g], axis=1).astype(ml_dtypes.bfloat16)
    woutb = w_out.astype(ml_dtypes.bfloat16)

    shared = dict(node=node_feats.astype(np.float32), wext=wext, wnode=wnode,
                  wout=woutb, lnw=lnw.astype(np.float32), lnb=lnb.astype(np.float32),
                  bg=b_g.astype(np.float32), bout=b_out.astype(np.float32))
    in_maps = []
    for c in range(n_cores):
        i0 = c * NI
        q = pair_feats[i0:i0 + NI].transpose(2, 0, 1)     # [C, NI, N]
        pt = q.reshape(C, NI, NJB, 128).transpose(0, 2, 1, 3)  # [C, jb, i, jj]
        in_maps.append(dict(
            pairt=np.ascontiguousarray(pt).astype(ml_dtypes.bfloat16),
            nodeq=np.ascontiguousarray(node_feats[i0:i0 + NI]).astype(np.float32),
            maskq=np.ascontiguousarray(mask[i0:i0 + NI]).astype(np.uint8),
            **shared))
    return in_maps


def split_sync_waits(nc, limit=1):
    """Walrus (this container's neuronxcc) rejects instructions carrying more
    than `limit` sem waits. Hoist excess waits onto per-engine carrier drains
    inserted just before the offending instruction."""
    n_split = 0
    for f in nc.m.functions:
        for bb in f.blocks:
            out = []
            for inst in bb.instructions:
                si = inst.sync_info
                waits = list(si.on_wait) if si and si.on_wait else []
                if len(waits) > limit:
                    extra, keep = waits[:-limit], waits[-limit:]
                    for ci in range(0, len(extra), limit):
                        chunk = extra[ci:ci+limit]
                        nd = mybir.InstDrain(name=f"{inst.name}-wsplit{ci}", ins=[], outs=[])
                        nd.engine = inst.engine
                        nd.sync_info = mybir.SyncInfo(on_wait=chunk, on_update=[])
                        out.append(nd)
                        n_split += 1
                    si.on_wait = keep
                out.append(inst)
            bb.instructions = out
    return n_split


_CACHED = {}


def kernel(**inputs):
    """Full-input entry point: shards over 8 NeuronCores, returns full output."""
    NC_CORES = 8
    NI = N // NC_CORES
    from concourse.bass_utils import run_bass_kernel_spmd

    in_maps = host_prep(inputs, NI=NI, n_cores=NC_CORES)
    if "nc" not in _CACHED:
        nc = build_nc(NI=NI, n_cores=NC_CORES)
        split_sync_waits(nc)
        _CACHED["nc"] = nc
    res = run_bass_kernel_spmd(_CACHED["nc"], in_maps, list(range(NC_CORES)))
    y = np.concatenate([res.results[c]["y"] for c in range(NC_CORES)], axis=0)
    return y[None].astype(np.float32)
